# revision 2
# baseline (speedup 1.0000x reference)
"""Masked multi-head attention (B=2, S=2048, E=1024, H=16, D=64) on 8 TRN2 cores.

Sharding: each core owns 2 heads (of 16) for BOTH batches.
  - QKV projections computed per-core for its 2 heads, all in transposed
    feature-major layout ([128 feat, 4096 tok]) with 512-wide moving streams;
    V is then transposed per 128-tile on the PE (identity matmul) into the
    token-major v_aug layout the PV matmul needs.
  - Attention: flash-style with transposed scores (scoresT[k, q] tiles),
    unsafe softmax (no max subtraction -- scores are ~N(0,1)), denominator
    accumulated via a leading ones-column prepended to V in the PV matmul.
    Diagonal k-tiles are trimmed to their valid q-range (less exp + less PE).
  - Exchange head-parallel -> slot-parallel via ONE AllToAll: core r's slot-s
    attention block goes only to core s (8x less traffic than the previous
    8 per-slot AllGathers, which saturated the CC stream and left the PE
    idle ~45us at the tail).
  - Output projection row-parallel over the received heads, bias fused via
    ScalarE activation; each core emits a transposed [1024, 512] slice;
    the host transposes + stacks.

Compute dtype bf16 (fp32 PSUM accumulation).
"""

import numpy as np
import ml_dtypes

BF16 = ml_dtypes.bfloat16

B, S, E, H, D = 2, 2048, 1024, 16, 64
P = 128
SG = B * S          # 4096 global sequence length (batch-major)
NKO = E // P        # 8 contraction tiles over E
NST = SG // P       # 32 seq tiles of 128
NSB = SG // 512     # 8 seq blocks of 512
QB = S // 512       # 4 q-blocks per batch

_built = None
LAST_RESULTS = None


def _build():
    global _built
    if _built is not None:
        return _built

    import concourse.bacc as bacc
    import concourse.mybir as mybir
    import concourse.tile as tile
    from concourse.masks import make_identity

    f32 = mybir.dt.float32
    bf16 = mybir.dt.bfloat16
    Exp = mybir.ActivationFunctionType.Exp
    Identity = mybir.ActivationFunctionType.Identity

    nc = bacc.Bacc("TRN2", target_bir_lowering=False, debug=False, num_devices=8)

    xT = nc.declare_dram_parameter("xT", [E, SG], bf16, isOutput=False)
    wq = nc.declare_dram_parameter("wq", [E, P], bf16, isOutput=False)
    wk = nc.declare_dram_parameter("wk", [E, P], bf16, isOutput=False)
    wv = nc.declare_dram_parameter("wv", [E, P], bf16, isOutput=False)
    wo = nc.declare_dram_parameter("wo", [E, E], bf16, isOutput=False)
    bo = nc.declare_dram_parameter("bo", [P, NKO], f32, isOutput=False)
    masks = nc.declare_dram_parameter("masks", [P, 2048], bf16, isOutput=False)
    outT = nc.declare_dram_parameter("outT", [E, 512], f32, isOutput=True)

    # AllToAll exchange: chunk s of a2a_in = my 2 heads' attention for slot s
    # (batch s//4, q-block s%4); chunk r of a2a_out = core r's heads for MY slot.
    a2a_in = nc.dram_tensor("a2a_in", [8, P, 512], bf16)
    a2a_out = nc.dram_tensor("a2a_out", [8, P, 512], bf16)

    with tile.TileContext(nc) as tc, \
         tc.tile_pool(name="const", bufs=1) as const:
        # ---- constant / persistent SBUF tensors ----
        xT_sb = const.tile([P, NKO, SG], bf16, name="xT_sb")
        wq_sb = const.tile([P, NKO, P], bf16, name="wq_sb")
        wk_sb = const.tile([P, NKO, P], bf16, name="wk_sb")
        wv_sb = const.tile([P, NKO, P], bf16, name="wv_sb")
        wo_sb = const.tile([P, NKO, E], bf16, name="wo_sb")
        bo_sb = const.tile([P, NKO], f32, name="bo_sb")
        masks_sb = const.tile([P, 2048], bf16, name="masks_sb")
        qT_sb = const.tile([P, SG], bf16, name="qT_sb")
        kT_sb = const.tile([P, SG], bf16, name="kT_sb")
        vT_sb = const.tile([P, SG], bf16, name="vT_sb")
        ident = const.tile([P, P], bf16, name="ident")
        # per seq-tile: [ones | v_h0(64) | ones | v_h1(64)] -- the leading ones
        # column makes the softmax denominator land on PSUM partition 0
        v_aug = const.tile([P, NST, 130], bf16, name="v_aug")

        # chunked loads (by seq-block) so the first projection block only
        # waits on its own 1MB slice of x
        nc.sync.dma_start(wq_sb, wq.rearrange("(ko p) m -> p ko m", p=P))
        nc.sync.dma_start(wk_sb, wk.rearrange("(ko p) m -> p ko m", p=P))
        nc.sync.dma_start(wv_sb, wv.rearrange("(ko p) m -> p ko m", p=P))
        xT_r = xT.rearrange("(ko p) s -> p ko s", p=P)
        for sb in range(NSB):
            nc.sync.dma_start(
                xT_sb[:, :, sb * 512:(sb + 1) * 512],
                xT_r[:, :, sb * 512:(sb + 1) * 512],
            )
        nc.sync.dma_start(masks_sb, masks[:])
        nc.sync.dma_start(bo_sb, bo[:])
        nc.sync.dma_start(wo_sb, wo.rearrange("(ko p) m -> p ko m", p=P))

        with tc.tile_pool(name="psBig", bufs=2, space="PSUM") as psBig, \
             tc.tile_pool(name="psSmall", bufs=4, space="PSUM") as psSmall, \
             tc.tile_pool(name="sb_att", bufs=3) as sba:
            nc.any.memset(v_aug[:, :, 0:1], 1.0)
            nc.any.memset(v_aug[:, :, 65:66], 1.0)
            make_identity(nc, ident)

            def proj_block(w_sb, dst, sb):
                ps = psBig.tile([P, 2, 512], f32, tag="big", name="ps_proj")
                for ko in range(NKO):
                    nc.tensor.matmul(
                        ps[:, 0, :],
                        w_sb[:, ko, :],
                        xT_sb[:, ko, sb * 512:(sb + 1) * 512],
                        start=(ko == 0),
                        stop=(ko == NKO - 1),
                    )
                nc.vector.tensor_copy(out=dst[:, sb * 512:(sb + 1) * 512], in_=ps[:, 0, :])

            def v_transpose(st):
                # PE-transpose one [128,128] tile of the feature-major V into
                # the token-major v_aug slot (heads split around ones cols)
                ps = psSmall.tile([P, P], bf16, tag="small", name="ps_vt")
                nc.tensor.transpose(ps, vT_sb[:, st * P:(st + 1) * P], ident)
                nc.vector.tensor_copy(
                    out=v_aug[:, st, 0:130].rearrange("p (h x) -> p h x", x=65)[:, :, 1:65],
                    in_=ps.rearrange("p (h x) -> p h x", x=64),
                )

            def attn_unit(b, qb):
                # both local heads; k-tiles in pairs -> wide exp ops.
                # Diagonal k-tiles (dj = kt - 4*qb >= 0) only touch q >= 128*dj.
                numer = [
                    psSmall.tile([65, 512], f32, tag="small", name="ps_nm_t")
                    for _ in range(2)
                ]
                nkt = 4 * qb + 4
                q0g = S * b + qb * 512  # global q start of this slot
                for kt0 in range(0, nkt, 2):
                    sc = [
                        psBig.tile([P, 2, 512], f32, tag="big", name="ps_sc_t")
                        for _ in range(2)
                    ]
                    ex = [
                        sba.tile([P, 2, 512], bf16, tag=f"exp{hl}", name="sb_ex_t")
                        for hl in range(2)
                    ]
                    # valid q-range start (within the 512 block) per j
                    qv = []
                    for j in range(2):
                        dj = kt0 + j - 4 * qb
                        qv.append(max(0, 128 * dj))
                    for j in range(2):
                        for hl in range(2):
                            nc.tensor.matmul(
                                sc[hl][:, j, qv[j]:512],
                                kT_sb[64 * hl:64 * hl + 64,
                                      S * b + (kt0 + j) * P:S * b + (kt0 + j + 1) * P],
                                qT_sb[64 * hl:64 * hl + 64, q0g + qv[j]:q0g + 512],
                                start=True,
                                stop=True,
                            )
                    if qv[0] == qv[1]:
                        # same trim (always 0 off-diagonal): one wide exp
                        for hl in range(2):
                            nc.scalar.activation(ex[hl], sc[hl], Exp, scale=0.125)
                    else:
                        for j in range(2):
                            for hl in range(2):
                                nc.scalar.activation(
                                    ex[hl][:, j, qv[j]:512],
                                    sc[hl][:, j, qv[j]:512],
                                    Exp, scale=0.125,
                                )
                    for j in range(2):
                        dj = kt0 + j - 4 * qb
                        if dj >= 0:
                            mrow = masks_sb[:, dj * 512 + qv[j]:(dj + 1) * 512]
                            for hl in range(2):
                                nc.vector.tensor_mul(
                                    out=ex[hl][:, j, qv[j]:512],
                                    in0=ex[hl][:, j, qv[j]:512],
                                    in1=mrow,
                                )
                    for j in range(2):
                        kt = kt0 + j
                        for hl in range(2):
                            nc.tensor.matmul(
                                numer[hl][:, qv[j]:512],
                                v_aug[:, 16 * b + kt, 65 * hl:65 * hl + 65],
                                ex[hl][:, j, qv[j]:512],
                                start=(kt == 0),
                                stop=(kt == nkt - 1),
                            )
                for hl in range(2):
                    recip = sba.tile([1, 512], f32, tag="recip", name="sb_rc_t")
                    nc.vector.reciprocal_approx_fast(recip, numer[hl][0:1, :])
                    rb = sba.tile([65, 512], f32, tag="rbcast", name="sb_rb_t")
                    nc.gpsimd.partition_broadcast(rb, recip)
                    attn = sba.tile([65, 512], bf16, tag="attn", name="sb_at_t")
                    nc.vector.tensor_mul(out=attn, in0=numer[hl][:, :], in1=rb)
                    nc.sync.dma_start(
                        a2a_in[4 * b + qb, 64 * hl:64 * hl + 64, :], attn[1:65, :]
                    )

            # batch-0 inputs first (q/k needed by scores, v by PV)
            for sb in range(4):
                proj_block(wq_sb, qT_sb, sb)
                proj_block(wk_sb, kT_sb, sb)
            for sb in range(4):
                proj_block(wv_sb, vT_sb, sb)
                for st in range(4 * sb, 4 * sb + 4):
                    v_transpose(st)

            # batch-0 attention interleaved with batch-1 projections
            a1 = []
            for sb in range(4, 8):
                a1.append(lambda sb=sb: proj_block(wq_sb, qT_sb, sb))
                a1.append(lambda sb=sb: proj_block(wk_sb, kT_sb, sb))
            for sb in range(4, 8):
                a1.append(lambda sb=sb: proj_block(wv_sb, vT_sb, sb))

                def _tp(sb=sb):
                    for st in range(4 * sb, 4 * sb + 4):
                        v_transpose(st)
                a1.append(_tp)
            for qb in range(QB):
                attn_unit(0, qb)
                take, a1 = a1[:4], a1[4:]
                for thunk in take:
                    thunk()
            for thunk in a1:
                thunk()
            for qb in range(QB):
                attn_unit(1, qb)

            # ---- exchange: one AllToAll; core r's slot-s block -> core s ----
            nc.gpsimd.collective_compute(
                "AllToAll",
                mybir.AluOpType.bypass,
                replica_groups=[list(range(8))],
                ins=[a2a_in[:].opt()],
                outs=[a2a_out[:].opt()],
            )

            # ---- output projection over the received heads ----
            attn_all = const.tile([P, 8, 512], bf16, name="attn_all")
            nc.sync.dma_start(attn_all, a2a_out.rearrange("c p f -> p c f"))
            out_sb = const.tile([P, NKO, 512], f32, name="out_sb")
            outT_r = outT.rearrange("(mo p) f -> p mo f", p=P)
            for mo in range(NKO):
                ps = psBig.tile([P, 2, 512], f32, tag="big", name="ps_out")
                for ci in range(8):
                    nc.tensor.matmul(
                        ps[:, 0, :],
                        wo_sb[:, ci, mo * P:(mo + 1) * P],
                        attn_all[:, ci, :],
                        start=(ci == 0),
                        stop=(ci == 7),
                    )
                nc.scalar.activation(
                    out_sb[:, mo, :], ps[:, 0, :], Identity,
                    bias=bo_sb[:, mo:mo + 1], scale=1.0,
                )
                nc.sync.dma_start(outT_r[:, mo:mo + 1, :], out_sb[:, mo:mo + 1, :])

    nc.compile()
    _built = nc
    return nc


def _host_masks():
    p = np.arange(P)[:, None]
    f = np.arange(512)[None, :]
    m = np.zeros((P, 4, 512), np.float32)
    for r in range(4):
        m[:, r, :] = (f >= P * r + p).astype(np.float32)
    return np.ascontiguousarray(m.reshape(P, 2048)).astype(BF16)


def kernel(**inputs):
    global LAST_RESULTS
    from concourse import bass_utils

    x = np.asarray(inputs["x"], np.float32)
    W_q = np.asarray(inputs["W_q"], np.float32)
    W_k = np.asarray(inputs["W_k"], np.float32)
    W_v = np.asarray(inputs["W_v"], np.float32)
    W_o = np.asarray(inputs["W_o"], np.float32)
    b_o = np.asarray(inputs["b_o"], np.float32)

    nc = _build()

    xT_all = np.ascontiguousarray(
        np.concatenate([x[0].T, x[1].T], axis=1)
    ).astype(BF16)
    wo_b = np.ascontiguousarray(W_o).astype(BF16)
    bo_t = np.ascontiguousarray(b_o.reshape(NKO, P).T).astype(np.float32)
    masks = _host_masks()

    in_maps = []
    for c in range(8):
        sl = slice(P * c, P * (c + 1))
        in_maps.append({
            "xT": xT_all,
            "wq": np.ascontiguousarray(W_q[:, sl]).astype(BF16),
            "wk": np.ascontiguousarray(W_k[:, sl]).astype(BF16),
            "wv": np.ascontiguousarray(W_v[:, sl]).astype(BF16),
            "wo": wo_b,
            "bo": bo_t,
            "masks": masks,
        })

    res = bass_utils.run_bass_kernel_spmd(nc, in_maps, core_ids=list(range(8)))
    LAST_RESULTS = res

    out = np.empty((B, S, E), np.float32)
    for c in range(8):
        b, qb = c // 4, c % 4
        out[b, 512 * qb:512 * (qb + 1), :] = np.asarray(
            res.results[c]["outT"], np.float32
        ).T
    return out
